# revision 24
# baseline (speedup 1.0000x reference)
"""ALiBi causal attention on 8 TRN2 NeuronCores.

Sharding: core c handles batch b = c//4 and global heads [4*(c%4), 4*(c%4)+4).
Attention is fully local per core; one 8-core AllToAll re-shards the attention
output (head-major -> token-major) for the output projection. Each core emits
512 output rows of its batch; host concatenates.

Host-side input prep: x and the weight slices are pre-transposed (d_model on
the partition axis) and cast to bf16, so the kernel starts matmuls straight
off the DMAs. Wo arrives as a (2048, 1024) "virtual" Wo.T with the other
batch-quad's feature rows zeroed, which makes the post-AllToAll output
projection identical on every core (SPMD) at the cost of a 2x contraction.

Score matmul trick: scores^T[j,i] = (q/8 . k)[j,i] + slope*j - slope*i is one
K=70 matmul: rows 0-63 head dims; kT rows 64-69 / qT rows 64-69 carry 3-term
bf16 decompositions of slope*j and -slope*i paired with ones. PSUM gets
scores+bias directly; exp is the only elementwise pass. V carries a ones
column so the PV matmul also emits the softmax denominator (output row 64).

Perf structure (vs the first working version):
- j-tiles processed in PAIRS sharing one (128, 1024) 2-bank PSUM score tile,
  halving ScalarE exp instruction count.
- softmax denominators inverted with reciprocal_approx_fast (single DVE op)
  and broadcast across partitions with a K=1 f32r outer-product matmul into
  the spare rows 64:128 of the pv PSUM tile -- gpsimd stays empty so the
  AllToAll triggers fire as soon as their inputs land.
- each AllToAll buffer lives in its own DRAM pool so the u=0 trigger does
  not conservatively wait on u=1's staging writes.
- the output projection is split into half-contractions: the fc-even half
  (fed by AllToAll 0) runs while AllToAll 1 is still in flight; the fc-odd
  half + combine run after.
"""

import sys

import numpy as np

try:
    import concourse  # noqa: F401
except ImportError:  # pragma: no cover
    sys.path.insert(0, "/opt/trn_rl_repo")

import ml_dtypes
from concourse import bacc, mybir
import concourse.tile as tile
from concourse.bass_utils import run_bass_kernel_spmd

BF16 = mybir.dt.bfloat16
F32 = mybir.dt.float32
F32R = mybir.dt.float32r

B, T, DM, H = 2, 2048, 1024, 16
D = DM // H            # 64 head dim
NCORES = 8
QUAD = 4               # cores per batch
HPC = 4                # heads per core
PB = 128               # partitions
IC = 512               # i-chunk (query cols per score tile)
JT = 128               # j-tile (key rows per score tile)
NTT = T // PB          # 16 token tiles
NDC = DM // PB         # 8 d_model chunks
FPC = HPC * D          # 256 features per core
TOUT = T // QUAD       # 512 output rows per core
NEG = -1.0e9

import os as _os
PSA = int(_os.environ.get("PSA", 2))
PSS = int(_os.environ.get("PSS", 2))   # score tiles are 2 banks each now
PSV = int(_os.environ.get("PSV", 2))
EPB = int(_os.environ.get("EPB", 4))

_cache = {}


def _build(sim=False, phase="full", expop=True, nheads=HPC):
    nc = bacc.Bacc("TRN2", target_bir_lowering=False, debug=False,
                   num_devices=NCORES)

    x_e = nc.dram_tensor("x", [DM, T], BF16, kind="ExternalInput")
    wq_e = nc.dram_tensor("wq", [DM, FPC], BF16, kind="ExternalInput")
    wk_e = nc.dram_tensor("wk", [DM, FPC], BF16, kind="ExternalInput")
    wv_e = nc.dram_tensor("wv", [DM, FPC], BF16, kind="ExternalInput")
    wo_e = nc.dram_tensor("wo", [DM, DM], BF16, kind="ExternalInput")
    mask_e = nc.dram_tensor("mask", [PB, PB], F32, kind="ExternalInput")
    kaug_e = nc.dram_tensor("kaug", [6 * HPC, T], BF16, kind="ExternalInput")
    qaug_e = nc.dram_tensor("qaug", [6 * HPC, T], BF16, kind="ExternalInput")
    fsel_e = nc.dram_tensor("fsel", [D, 2], F32, kind="ExternalInput")
    out_e = nc.dram_tensor("out", [TOUT, DM], F32, kind="ExternalOutput")

    from contextlib import ExitStack
    with tile.TileContext(nc) as tc, ExitStack() as es:
            def pool(**kw):
                return es.enter_context(tc.tile_pool(**kw))
            xtp = pool(name="xt", bufs=8)          # xT chunks
            wtp = pool(name="wt", bufs=24)         # WqkvT chunks
            wop = pool(name="wo", bufs=16)         # WoT virtual
            qkp = pool(name="qk", bufs=8)          # qT/kT (70,T)
            vp = pool(name="vp", bufs=64)          # v tiles (128,65)
            smp = pool(name="small", bufs=2)       # misc small
            rcp = pool(name="rcp", bufs=2)         # recip rows
            bcp = pool(name="bcp", bufs=2)         # broadcast recip
            ep = pool(name="ep", bufs=EPB)         # exp tiles
            op = pool(name="op", bufs=4)           # outT tiles
            gp = pool(name="gp", bufs=8)           # gathered halves
            pop = pool(name="po", bufs=4)          # partial A out
            fo = pool(name="fo", bufs=2)           # final out stage
            psA = pool(name="psA", bufs=PSA, space="PSUM")  # proj
            psS = pool(name="psS", bufs=PSS, space="PSUM")  # score
            psV = pool(name="psV", bufs=PSV, space="PSUM")  # pv
            dpi0 = pool(name="dpi0", bufs=1, space="DRAM")
            dpo0 = pool(name="dpo0", bufs=1, space="DRAM")
            dpi2 = pool(name="dpi2", bufs=1, space="DRAM")
            dpo2 = pool(name="dpo2", bufs=1, space="DRAM")
            dpi3 = pool(name="dpi3", bufs=1, space="DRAM")
            dpo3 = pool(name="dpo3", bufs=1, space="DRAM")

            # ---- constants ----
            mask = smp.tile([PB, PB], F32, tag="mask")
            nc.sync.dma_start(out=mask[:, :], in_=mask_e[:, :])
            fsel = smp.tile([D, 2], F32, tag="fsel")
            nc.sync.dma_start(out=fsel[:, :], in_=fsel_e[:, :])


            # ---- input DMAs. dma_start issue costs ~0.65us of serial engine
            # time, so spread issues over sync/vector/scalar and order them
            # so the first qk_proj matmul group (wq + x token-chunk 0) is
            # ready first.
            xT = [xtp.tile([PB, T], BF16, tag="xt", name=f"xT{dc}")
                  for dc in range(NDC)]
            wT = {}
            for wi, w_e in enumerate((wq_e, wk_e, wv_e)):
                wT[wi] = [wtp.tile([PB, FPC], BF16, tag="wt",
                                   name=f"wT{wi}_{dc}")
                          for dc in range(NDC)]
            for dc in range(NDC):
                nc.sync.dma_start(out=wT[0][dc][:, :],
                                  in_=wq_e[dc * PB:(dc + 1) * PB, :])
                nc.scalar.dma_start(out=xT[dc][:, 0:IC],
                                    in_=x_e[dc * PB:(dc + 1) * PB, 0:IC])
            for dc in range(NDC):
                nc.gpsimd.dma_start(out=wT[1][dc][:, :],
                                    in_=wk_e[dc * PB:(dc + 1) * PB, :])
                nc.sync.dma_start(out=xT[dc][:, IC:T],
                                  in_=x_e[dc * PB:(dc + 1) * PB, IC:T])
                nc.gpsimd.dma_start(out=wT[2][dc][:, :],
                                    in_=wv_e[dc * PB:(dc + 1) * PB, :])

            # ---- projections ----
            # qTt[l]/kTt[l]: (70, T); rows 0-63 data, 64-69 aug rows.
            qTt = [qkp.tile([70, T], BF16, tag="qk", name=f"qT{l}")
                   for l in range(HPC)]
            kTt = [qkp.tile([70, T], BF16, tag="qk", name=f"kT{l}")
                   for l in range(HPC)]
            for l in range(HPC):
                nc.sync.dma_start(out=kTt[l][64:70, :],
                                  in_=kaug_e[6 * l:6 * l + 6, :])
                nc.sync.dma_start(out=qTt[l][64:70, :],
                                  in_=qaug_e[6 * l:6 * l + 6, :])

            # q, k: out (128 f = 2 heads, 512 t) accumulated over d chunks
            def qk_proj(fb):
                for wi, dest, scl in ((0, qTt, 0.125), (1, kTt, 1.0)):
                    for tch in range(T // IC):
                        pp = psA.tile([PB, IC], F32, tag="pp",
                                      name=f"qk{wi}{fb}{tch}")
                        for dc in range(NDC):
                            nc.tensor.matmul(
                                pp[:, :],
                                wT[wi][dc][:, fb * PB:(fb + 1) * PB],
                                xT[dc][:, tch * IC:(tch + 1) * IC],
                                start=(dc == 0), stop=(dc == NDC - 1))
                        for hh in range(2):  # split head pair
                            l = 2 * fb + hh
                            dst = dest[l][0:64, tch * IC:(tch + 1) * IC]
                            if tch % 2 == 0:
                                nc.scalar.mul(dst, pp[hh * D:(hh + 1) * D, :],
                                              scl)
                            else:
                                nc.vector.tensor_scalar_mul(
                                    dst, pp[hh * D:(hh + 1) * D, :], scl)
            qk_proj(0)

            # v natural: (128 t, 256 f) accumulated over d chunks; split into
            # per-head (128, 65) tiles with a ones column at col 64.
            vt = {}
            for l in range(HPC):
                vt[l] = [vp.tile([PB, D + 1], BF16, tag="vp",
                                 name=f"v{l}_{tt}")
                         for tt in range(NTT)]
            for tt in range(NTT):
                pp = psA.tile([PB, FPC], F32, tag="pp")
                for dc in range(NDC):
                    nc.tensor.matmul(pp[:, :],
                                     xT[dc][:, tt * PB:(tt + 1) * PB],
                                     wT[2][dc][:, :],
                                     start=(dc == 0), stop=(dc == NDC - 1))
                for l in range(HPC):
                    if l % 2 == 0:
                        nc.scalar.copy(vt[l][tt][:, 0:D],
                                       pp[:, l * D:(l + 1) * D])
                    else:
                        nc.vector.tensor_copy(vt[l][tt][:, 0:D],
                                              pp[:, l * D:(l + 1) * D])
                    nc.vector.memset(vt[l][tt][:, D:D + 1], 1.0)

            if phase == "proj":
                fot0 = fo.tile([PB, IC], F32, tag="fo")
                nc.vector.tensor_copy(fot0[0:64, :].bitcast(BF16),
                                      qTt[0][0:64, 0:1024])
                for l in range(HPC):
                    nc.vector.tensor_copy(
                        fot0[64:128, :].bitcast(BF16),
                        kTt[l][0:64, 0:1024])
                    nc.vector.tensor_copy(
                        fot0[0:128, 0:32].bitcast(BF16), vt[l][0][:, 0:64])
                nc.sync.dma_start(out=out_e[0:PB, 0:IC], in_=fot0[:, :])

            # ---- WoT chunks (issued early; needed only at the tail) ----
            woT = []
            for fc in range(DM // PB if phase == "full" else 0):
                t_ = wop.tile([PB, DM], BF16, tag="wo", name=f"woT{fc}")
                nc.scalar.dma_start(out=t_[:, :],
                                    in_=wo_e[fc * PB:(fc + 1) * PB, :])
                woT.append(t_)

            # ---- attention (head-pair outer, i-chunk inner) ----
            # heads 0-1 ride one paired 1MB AllToAll (hidden under heads
            # 2-3); heads 2 and 3 each get their own 512KB AllToAll so only
            # head 3's collective is exposed in the tail.
            a2a_in0 = dpi0.tile([NCORES, PB, TOUT], BF16, tag="a2ain0",
                                name="a2ai0")
            a2a_out0 = dpo0.tile([NCORES, PB, TOUT], BF16, tag="a2aout0",
                                 name="a2ao0")
            a2a_inh = {2: dpi2.tile([NCORES, D, TOUT], BF16, tag="a2ain2",
                                    name="a2ai2"),
                       3: dpi3.tile([NCORES, D, TOUT], BF16, tag="a2ain3",
                                    name="a2ai3")}
            a2a_outh = {2: dpo2.tile([NCORES, D, TOUT], BF16, tag="a2aout2",
                                     name="a2ao2"),
                        3: dpo3.tile([NCORES, D, TOUT], BF16, tag="a2aout3",
                                     name="a2ao3")}

            def a2a(in_t, out_t):
                if sim:
                    nc.gpsimd.dma_start(out=out_t[:, :, :],
                                        in_=in_t[:, :, :])
                else:
                    nc.gpsimd.collective_compute(
                        "AllToAll", mybir.AluOpType.bypass,
                        replica_groups=[list(range(NCORES))],
                        ins=[in_t.opt()], outs=[out_t.opt()])

            oTs = {}
            oTs2 = {}
            if phase == "proj":
                qk_proj(1)
            for l in range(nheads if phase != "proj" else 0):
                if l == 2:
                    qk_proj(1)
                for ichk in range(T // IC):
                    i0 = ichk * IC
                    njt = i0 // JT + 4           # j-tiles for this i-chunk
                    pv = psV.tile([D + 1, IC], F32, tag="pv")
                    # j-tiles in pairs sharing one 2-bank score tile
                    for jp in range(0, njt, 2):
                        jts = list(range(jp, min(jp + 2, njt)))
                        spp = psS.tile([PB, 2 * IC], F32, tag="sp")
                        et = ep.tile([PB, 2 * IC], BF16, tag="ep")
                        nns = []
                        for h, jt in enumerate(jts):
                            j0 = jt * JT
                            ist = max(i0, j0)    # trim: only i >= j0
                            nn = IC - (ist - i0)
                            nns.append(nn)
                            nc.tensor.matmul(
                                spp[:, h * IC:h * IC + nn],
                                kTt[l][:, j0:j0 + JT],
                                qTt[l][:, ist:i0 + IC],
                                start=True, stop=True)
                            if j0 >= i0:         # diagonal tile: causal mask
                                nc.vector.tensor_add(
                                    spp[:, h * IC:h * IC + JT],
                                    spp[:, h * IC:h * IC + JT], mask[:, :])
                        # exp over contiguous valid spans (no stale reads)
                        if expop:
                            if len(jts) == 2 and nns[0] == IC:
                                nc.scalar.activation(
                                    et[:, 0:IC + nns[1]],
                                    spp[:, 0:IC + nns[1]],
                                    mybir.ActivationFunctionType.Exp)
                            else:
                                for h, jt in enumerate(jts):
                                    nc.scalar.activation(
                                        et[:, h * IC:h * IC + nns[h]],
                                        spp[:, h * IC:h * IC + nns[h]],
                                        mybir.ActivationFunctionType.Exp)
                        else:
                            for h, jt in enumerate(jts):
                                nc.scalar.copy(et[:, h * IC:h * IC + nns[h]],
                                               spp[:, h * IC:h * IC + nns[h]])
                        for h, jt in enumerate(jts):
                            noff = IC - nns[h]
                            nc.tensor.matmul(
                                pv[0:D + 1, noff:IC],
                                vt[l][jt][:, :],
                                et[:, h * IC:h * IC + nns[h]],
                                start=(jt == 0), stop=(jt == njt - 1))
                    # normalize: copy denom row off partition 64, fast recip
                    # (custom-DVE op needs partition-0 operands), then DMA
                    # partition-broadcast into SBUF. The broadcast must NOT
                    # ride gpsimd: that in-order queue holds the collective
                    # triggers, which would then fire only after every
                    # queued broadcast (measured +50us on the tail).
                    dn = rcp.tile([1, IC], F32, tag="dn")
                    nc.vector.tensor_copy(dn[:, :], pv[D:D + 1, :])
                    rc = rcp.tile([1, IC], F32, tag="rc")
                    nc.vector.reciprocal_approx_fast(out=rc[:, :],
                                                     in_=dn[:, :])
                    bcs = bcp.tile([D, IC], F32, tag="bcs")
                    nc.sync.dma_start(
                        out=bcs[:, :],
                        in_=rc[0:1, None, :].broadcast_to([1, D, IC]))
                    # payload x own-quad flag to each slot pair: receivers
                    # sum chunk pairs, so quad selection happens in the data
                    # (fsel per-core constant), keeping the program SPMD
                    if l < 2:
                        r = (l % 2) * D
                        if r == 0:
                            oTs[ichk] = op.tile([PB, IC], BF16, tag="opa",
                                                name=f"oTa{ichk}")
                            oTs2[ichk] = op.tile([PB, IC], BF16, tag="opb",
                                                 name=f"oTb{ichk}")
                        nc.vector.scalar_tensor_tensor(
                            oTs[ichk][r:r + D, :], pv[0:D, :], fsel[:, 0:1],
                            bcs[:, :], mybir.AluOpType.mult,
                            mybir.AluOpType.mult)
                        nc.vector.scalar_tensor_tensor(
                            oTs2[ichk][r:r + D, :], pv[0:D, :], fsel[:, 1:2],
                            bcs[:, :], mybir.AluOpType.mult,
                            mybir.AluOpType.mult)
                        if r != 0 or nheads == 1:
                            nc.sync.dma_start(
                                out=a2a_in0[ichk:ichk + 1, :, :],
                                in_=oTs[ichk][:, :])
                            nc.sync.dma_start(
                                out=a2a_in0[ichk + 4:ichk + 5, :, :],
                                in_=oTs2[ichk][:, :])
                    else:
                        oha = op.tile([D, IC], BF16, tag="oha",
                                      name=f"oha{ichk}_{l}")
                        ohb = op.tile([D, IC], BF16, tag="ohb",
                                      name=f"ohb{ichk}_{l}")
                        nc.vector.scalar_tensor_tensor(
                            oha[:, :], pv[0:D, :], fsel[:, 0:1],
                            bcs[:, :], mybir.AluOpType.mult,
                            mybir.AluOpType.mult)
                        nc.vector.scalar_tensor_tensor(
                            ohb[:, :], pv[0:D, :], fsel[:, 1:2],
                            bcs[:, :], mybir.AluOpType.mult,
                            mybir.AluOpType.mult)
                        nc.sync.dma_start(
                            out=a2a_inh[l][ichk:ichk + 1, :, :],
                            in_=oha[:, :])
                        nc.sync.dma_start(
                            out=a2a_inh[l][ichk + 4:ichk + 5, :, :],
                            in_=ohb[:, :])
                if phase == "full":
                    if l == 1 or (l == 0 and nheads == 1):
                        a2a(a2a_in0, a2a_out0)
                    elif l >= 2:
                        a2a(a2a_inh[l], a2a_outh[l])
            if phase == "attn":
                nc.gpsimd.dma_start(out=a2a_out0[0:1, :, :],
                                    in_=a2a_in0[0:1, :, :])

            if phase != "full":
                fob = fo.tile([PB, DM], F32, tag="fo")
                if phase == "attn":
                    nc.sync.dma_start(out=fob[:, 0:256].bitcast(BF16),
                                      in_=a2a_out0[0:1, :, :])
                else:
                    nc.vector.memset(fob[:, :], 0.0)
                nc.sync.dma_start(out=out_e[PB:2 * PB, :], in_=fob[:, :])

            # ---- gather + output projection ----
            # g0[p]: heads {4p, 4p+1} (from the paired AllToAll); partial A
            # runs while head 3's AllToAll is in flight. g23[p] stacks head
            # 4p+2 (rows 0:64, from A2A h2) on head 4p+3 (rows 64:128, A2A
            # h3) -- matching WoT's odd 128-row chunks, so partial B is 4
            # full K=128 matmuls per psum tile.
            if phase == "full":
                g0 = {}
                for p in range(4):
                    ga = gp.tile([PB, TOUT], BF16, tag="gab", name=f"ga{p}")
                    gb = gp.tile([PB, TOUT], BF16, tag="gab", name=f"gb{p}")
                    nc.sync.dma_start(out=ga[:, :],
                                      in_=a2a_out0[p:p + 1, :, :])
                    nc.sync.dma_start(out=gb[:, :],
                                      in_=a2a_out0[p + 4:p + 5, :, :])
                    gs = gp.tile([PB, TOUT], BF16, tag="gs", name=f"gs{p}")
                    nc.vector.tensor_add(gs[:, :], ga[:, :], gb[:, :])
                    g0[p] = gs
                pots = {}
                for tt4 in range(4):
                    pot = pop.tile([PB, DM], F32, tag="po", name=f"po{tt4}")
                    for oc in range(2):
                        pp = psA.tile([PB, IC], F32, tag="pp")
                        for p in range(4):
                            nc.tensor.matmul(
                                pp[:, :],
                                g0[p][:, tt4 * PB:(tt4 + 1) * PB],
                                woT[2 * p][:, oc * IC:(oc + 1) * IC],
                                start=(p == 0), stop=(p == 3))
                        if oc == 0:
                            nc.scalar.copy(pot[:, 0:IC], pp[:, :])
                        else:
                            nc.vector.tensor_copy(pot[:, IC:DM], pp[:, :])
                    pots[tt4] = pot
                g23 = {}
                for p in range(4):
                    ga = gp.tile([PB, TOUT], BF16, tag="gab", name=f"gha{p}")
                    gb = gp.tile([PB, TOUT], BF16, tag="gab", name=f"ghb{p}")
                    nc.sync.dma_start(out=ga[0:D, :],
                                      in_=a2a_outh[2][p:p + 1, :, :])
                    nc.sync.dma_start(out=ga[D:PB, :],
                                      in_=a2a_outh[3][p:p + 1, :, :])
                    nc.sync.dma_start(out=gb[0:D, :],
                                      in_=a2a_outh[2][p + 4:p + 5, :, :])
                    nc.sync.dma_start(out=gb[D:PB, :],
                                      in_=a2a_outh[3][p + 4:p + 5, :, :])
                    gs = gp.tile([PB, TOUT], BF16, tag="gs", name=f"gh{p}")
                    nc.vector.tensor_add(gs[:, :], ga[:, :], gb[:, :])
                    g23[p] = gs
                for tt4 in range(4):
                    fot = fo.tile([PB, DM], F32, tag="fo")
                    for oc in range(2):
                        pp = psA.tile([PB, IC], F32, tag="pp")
                        for p in range(4):
                            nc.tensor.matmul(
                                pp[:, :],
                                g23[p][:, tt4 * PB:(tt4 + 1) * PB],
                                woT[2 * p + 1][:, oc * IC:(oc + 1) * IC],
                                start=(p == 0), stop=(p == 3))
                        nc.vector.tensor_add(fot[:, oc * IC:(oc + 1) * IC],
                                             pp[:, :],
                                             pots[tt4][:, oc * IC:(oc + 1) * IC])
                    nc.sync.dma_start(out=out_e[tt4 * PB:(tt4 + 1) * PB, :],
                                      in_=fot[:, :])

    nc.compile()
    return nc


def _build_env():
    import os
    return _build(sim=bool(os.environ.get("NO_COLL")),
                  phase=os.environ.get("PHASE", "full"),
                  expop=not os.environ.get("NO_EXP"),
                  nheads=int(os.environ.get("NHEADS", HPC)))


def _consts(m):
    """Per-core constant tensors; m = core % 4 (quad rank)."""
    bf = ml_dtypes.bfloat16

    def dec3(v):
        hi = v.astype(bf).astype(np.float32)
        mid = (v - hi).astype(bf).astype(np.float32)
        lo = (v - hi - mid).astype(bf).astype(np.float32)
        return hi, mid, lo

    heads = [4 * m + l for l in range(HPC)]
    slopes = [2.0 ** (-8.0 * (g + 1) / H) for g in heads]
    pos = np.arange(T, dtype=np.float32)
    kaug = np.zeros((6 * HPC, T), np.float32)
    qaug = np.zeros((6 * HPC, T), np.float32)
    for l, s in enumerate(slopes):
        kaug[6 * l:6 * l + 3] = dec3(s * pos)    # slope * j, 3-term exact
        kaug[6 * l + 3:6 * l + 6] = 1.0
        qaug[6 * l:6 * l + 3] = 1.0
        qaug[6 * l + 3:6 * l + 6] = dec3(-s * pos)  # -slope * i
    mask = np.where(np.arange(PB)[None, :] >= np.arange(PB)[:, None],
                    0.0, NEG).astype(np.float32)  # mask[jp, c]: c >= jp valid
    fsel = np.zeros((D, 2), np.float32)
    return dict(mask=mask, kaug=kaug.astype(bf), qaug=qaug.astype(bf),
                fsel=fsel)


def _in_maps(x, Wq, Wk, Wv, Wo):
    bf = ml_dtypes.bfloat16
    x = np.asarray(x, np.float32)
    xTb = [np.ascontiguousarray(x[b].T).astype(bf) for b in range(B)]
    WqT = np.asarray(Wq, np.float32).T.astype(bf)   # (DM in, DM features)
    WkT = np.asarray(Wk, np.float32).T.astype(bf)
    WvT = np.asarray(Wv, np.float32).T.astype(bf)
    WoT = np.asarray(Wo, np.float32).T.astype(bf)   # (DM f, DM o)
    maps = []
    for c in range(NCORES):
        b, m = c // QUAD, c % QUAD
        fs = FPC * m
        mp = dict(x=xTb[b],
                  wq=np.ascontiguousarray(WqT[:, fs:fs + FPC]),
                  wk=np.ascontiguousarray(WkT[:, fs:fs + FPC]),
                  wv=np.ascontiguousarray(WvT[:, fs:fs + FPC]),
                  wo=WoT, **_consts(m))
        mp["fsel"][:, b] = 1.0
        maps.append(mp)
    return maps


def _assemble(results):
    out = np.zeros((B, T, DM), np.float32)
    for c in range(NCORES):
        b, m = c // QUAD, c % QUAD
        out[b, m * TOUT:(m + 1) * TOUT, :] = results[c]["out"]
    return out


def get_nc():
    if "nc" not in _cache:
        _cache["nc"] = _build()
    return _cache["nc"]


def run(inputs, trace=False, **kw):
    nc = get_nc()
    maps = _in_maps(**inputs)
    res = run_bass_kernel_spmd(nc, maps, core_ids=list(range(NCORES)),
                               trace=trace, **kw)
    return _assemble(res.results), res


def kernel(x, Wq, Wk, Wv, Wo):
    out, _ = run(dict(x=x, Wq=Wq, Wk=Wk, Wv=Wv, Wo=Wo))
    return out


# revision 25
# speedup vs baseline: 1.0190x; 1.0190x over previous
"""ALiBi causal attention on 8 TRN2 NeuronCores.

Sharding: core c handles batch b = c//4 and global heads [4*(c%4), 4*(c%4)+4).
Attention is fully local per core; one 8-core AllToAll re-shards the attention
output (head-major -> token-major) for the output projection. Each core emits
512 output rows of its batch; host concatenates.

Host-side input prep: x and the weight slices are pre-transposed (d_model on
the partition axis) and cast to bf16, so the kernel starts matmuls straight
off the DMAs. Wo arrives as a (2048, 1024) "virtual" Wo.T with the other
batch-quad's feature rows zeroed, which makes the post-AllToAll output
projection identical on every core (SPMD) at the cost of a 2x contraction.

Score matmul trick: scores^T[j,i] = (q/8 . k)[j,i] + slope*j - slope*i is one
K=70 matmul: rows 0-63 head dims; kT rows 64-69 / qT rows 64-69 carry 3-term
bf16 decompositions of slope*j and -slope*i paired with ones. PSUM gets
scores+bias directly; exp is the only elementwise pass. V carries a ones
column so the PV matmul also emits the softmax denominator (output row 64).

Perf structure (vs the first working version):
- j-tiles processed in PAIRS sharing one (128, 1024) 2-bank PSUM score tile,
  halving ScalarE exp instruction count.
- softmax denominators inverted with reciprocal_approx_fast (single DVE op)
  and broadcast across partitions with a K=1 f32r outer-product matmul into
  the spare rows 64:128 of the pv PSUM tile -- gpsimd stays empty so the
  AllToAll triggers fire as soon as their inputs land.
- each AllToAll buffer lives in its own DRAM pool so the u=0 trigger does
  not conservatively wait on u=1's staging writes.
- the output projection is split into half-contractions: the fc-even half
  (fed by AllToAll 0) runs while AllToAll 1 is still in flight; the fc-odd
  half + combine run after.
"""

import sys

import numpy as np

try:
    import concourse  # noqa: F401
except ImportError:  # pragma: no cover
    sys.path.insert(0, "/opt/trn_rl_repo")

import ml_dtypes
from concourse import bacc, mybir
import concourse.tile as tile
from concourse.bass_utils import run_bass_kernel_spmd

BF16 = mybir.dt.bfloat16
F32 = mybir.dt.float32
F32R = mybir.dt.float32r

B, T, DM, H = 2, 2048, 1024, 16
D = DM // H            # 64 head dim
NCORES = 8
QUAD = 4               # cores per batch
HPC = 4                # heads per core
PB = 128               # partitions
IC = 512               # i-chunk (query cols per score tile)
JT = 128               # j-tile (key rows per score tile)
NTT = T // PB          # 16 token tiles
NDC = DM // PB         # 8 d_model chunks
FPC = HPC * D          # 256 features per core
TOUT = T // QUAD       # 512 output rows per core
NEG = -1.0e9

import os as _os
PSA = int(_os.environ.get("PSA", 2))
PSS = int(_os.environ.get("PSS", 2))   # score tiles are 2 banks each now
PSV = int(_os.environ.get("PSV", 2))
EPB = int(_os.environ.get("EPB", 4))

_cache = {}


def _build(sim=False, phase="full", expop=True, nheads=HPC):
    nc = bacc.Bacc("TRN2", target_bir_lowering=False, debug=False,
                   num_devices=NCORES)

    x_e = nc.dram_tensor("x", [DM, T], BF16, kind="ExternalInput")
    wq_e = nc.dram_tensor("wq", [DM, FPC], BF16, kind="ExternalInput")
    wk_e = nc.dram_tensor("wk", [DM, FPC], BF16, kind="ExternalInput")
    wv_e = nc.dram_tensor("wv", [DM, FPC], BF16, kind="ExternalInput")
    wo_e = nc.dram_tensor("wo", [DM, DM], BF16, kind="ExternalInput")
    mask_e = nc.dram_tensor("mask", [PB, PB], F32, kind="ExternalInput")
    kaug_e = nc.dram_tensor("kaug", [6 * HPC, T], BF16, kind="ExternalInput")
    qaug_e = nc.dram_tensor("qaug", [6 * HPC, T], BF16, kind="ExternalInput")
    fsel_e = nc.dram_tensor("fsel", [D, 2], F32, kind="ExternalInput")
    out_e = nc.dram_tensor("out", [TOUT, DM], F32, kind="ExternalOutput")

    from contextlib import ExitStack
    with tile.TileContext(nc) as tc, ExitStack() as es:
            def pool(**kw):
                return es.enter_context(tc.tile_pool(**kw))
            xtp = pool(name="xt", bufs=8)          # xT chunks
            wtp = pool(name="wt", bufs=24)         # WqkvT chunks
            wop = pool(name="wo", bufs=16)         # WoT virtual
            qkp = pool(name="qk", bufs=8)          # qT/kT (70,T)
            vp = pool(name="vp", bufs=64)          # v tiles (128,65)
            smp = pool(name="small", bufs=2)       # misc small
            rcp = pool(name="rcp", bufs=2)         # recip rows
            bcp = pool(name="bcp", bufs=2)         # broadcast recip
            ep = pool(name="ep", bufs=EPB)         # exp tiles
            op = pool(name="op", bufs=4)           # outT tiles
            gp = pool(name="gp", bufs=8)           # gathered halves
            pop = pool(name="po", bufs=4)          # partial A out
            fo = pool(name="fo", bufs=2)           # final out stage
            psA = pool(name="psA", bufs=PSA, space="PSUM")  # proj
            psS = pool(name="psS", bufs=PSS, space="PSUM")  # score
            psV = pool(name="psV", bufs=PSV, space="PSUM")  # pv
            dpi0 = pool(name="dpi0", bufs=1, space="DRAM")
            dpo0 = pool(name="dpo0", bufs=1, space="DRAM")
            dpi2 = pool(name="dpi2", bufs=1, space="DRAM")
            dpo2 = pool(name="dpo2", bufs=1, space="DRAM")
            dpi3 = pool(name="dpi3", bufs=1, space="DRAM")
            dpo3 = pool(name="dpo3", bufs=1, space="DRAM")

            # ---- constants ----
            mask = smp.tile([PB, PB], F32, tag="mask")
            nc.sync.dma_start(out=mask[:, :], in_=mask_e[:, :])
            fsel = smp.tile([D, 2], F32, tag="fsel")
            nc.sync.dma_start(out=fsel[:, :], in_=fsel_e[:, :])


            # ---- input DMAs. dma_start issue costs ~0.65us of serial engine
            # time, so spread issues over sync/vector/scalar and order them
            # so the first qk_proj matmul group (wq + x token-chunk 0) is
            # ready first.
            xT = [xtp.tile([PB, T], BF16, tag="xt", name=f"xT{dc}")
                  for dc in range(NDC)]
            wT = {}
            for wi, w_e in enumerate((wq_e, wk_e, wv_e)):
                wT[wi] = [wtp.tile([PB, FPC], BF16, tag="wt",
                                   name=f"wT{wi}_{dc}")
                          for dc in range(NDC)]
            for dc in range(NDC):
                nc.sync.dma_start(out=wT[0][dc][:, :],
                                  in_=wq_e[dc * PB:(dc + 1) * PB, :])
                nc.scalar.dma_start(out=xT[dc][:, 0:IC],
                                    in_=x_e[dc * PB:(dc + 1) * PB, 0:IC])
            for dc in range(NDC):
                nc.scalar.dma_start(out=wT[1][dc][:, :],
                                    in_=wk_e[dc * PB:(dc + 1) * PB, :])
                nc.sync.dma_start(out=xT[dc][:, IC:T],
                                  in_=x_e[dc * PB:(dc + 1) * PB, IC:T])
                nc.sync.dma_start(out=wT[2][dc][:, :],
                                  in_=wv_e[dc * PB:(dc + 1) * PB, :])

            # ---- projections ----
            # qTt[l]/kTt[l]: (70, T); rows 0-63 data, 64-69 aug rows.
            qTt = [qkp.tile([70, T], BF16, tag="qk", name=f"qT{l}")
                   for l in range(HPC)]
            kTt = [qkp.tile([70, T], BF16, tag="qk", name=f"kT{l}")
                   for l in range(HPC)]
            for l in range(HPC):
                nc.sync.dma_start(out=kTt[l][64:70, :],
                                  in_=kaug_e[6 * l:6 * l + 6, :])
                nc.sync.dma_start(out=qTt[l][64:70, :],
                                  in_=qaug_e[6 * l:6 * l + 6, :])

            # q, k: out (128 f = 2 heads, 512 t) accumulated over d chunks
            def qk_proj(fb):
                for wi, dest, scl in ((0, qTt, 0.125), (1, kTt, 1.0)):
                    for tch in range(T // IC):
                        pp = psA.tile([PB, IC], F32, tag="pp",
                                      name=f"qk{wi}{fb}{tch}")
                        for dc in range(NDC):
                            nc.tensor.matmul(
                                pp[:, :],
                                wT[wi][dc][:, fb * PB:(fb + 1) * PB],
                                xT[dc][:, tch * IC:(tch + 1) * IC],
                                start=(dc == 0), stop=(dc == NDC - 1))
                        for hh in range(2):  # split head pair
                            l = 2 * fb + hh
                            dst = dest[l][0:64, tch * IC:(tch + 1) * IC]
                            if tch % 2 == 0:
                                nc.scalar.mul(dst, pp[hh * D:(hh + 1) * D, :],
                                              scl)
                            else:
                                nc.vector.tensor_scalar_mul(
                                    dst, pp[hh * D:(hh + 1) * D, :], scl)
            qk_proj(0)

            # v natural: (128 t, 256 f) accumulated over d chunks; split into
            # per-head (128, 65) tiles with a ones column at col 64.
            vt = {}
            for l in range(HPC):
                vt[l] = [vp.tile([PB, D + 1], BF16, tag="vp",
                                 name=f"v{l}_{tt}")
                         for tt in range(NTT)]
            for tt in range(NTT):
                pp = psA.tile([PB, FPC], F32, tag="pp")
                for dc in range(NDC):
                    nc.tensor.matmul(pp[:, :],
                                     xT[dc][:, tt * PB:(tt + 1) * PB],
                                     wT[2][dc][:, :],
                                     start=(dc == 0), stop=(dc == NDC - 1))
                for l in range(HPC):
                    if l % 2 == 0:
                        nc.scalar.copy(vt[l][tt][:, 0:D],
                                       pp[:, l * D:(l + 1) * D])
                    else:
                        nc.vector.tensor_copy(vt[l][tt][:, 0:D],
                                              pp[:, l * D:(l + 1) * D])
                    nc.vector.memset(vt[l][tt][:, D:D + 1], 1.0)

            if phase == "proj":
                fot0 = fo.tile([PB, IC], F32, tag="fo")
                nc.vector.tensor_copy(fot0[0:64, :].bitcast(BF16),
                                      qTt[0][0:64, 0:1024])
                for l in range(HPC):
                    nc.vector.tensor_copy(
                        fot0[64:128, :].bitcast(BF16),
                        kTt[l][0:64, 0:1024])
                    nc.vector.tensor_copy(
                        fot0[0:128, 0:32].bitcast(BF16), vt[l][0][:, 0:64])
                nc.sync.dma_start(out=out_e[0:PB, 0:IC], in_=fot0[:, :])

            # ---- WoT chunks (issued early; needed only at the tail) ----
            woT = []
            for fc in range(DM // PB if phase == "full" else 0):
                t_ = wop.tile([PB, DM], BF16, tag="wo", name=f"woT{fc}")
                nc.scalar.dma_start(out=t_[:, :],
                                    in_=wo_e[fc * PB:(fc + 1) * PB, :])
                woT.append(t_)

            # ---- attention (head-pair outer, i-chunk inner) ----
            # heads 0-1 ride one paired 1MB AllToAll (hidden under heads
            # 2-3); heads 2 and 3 each get their own 512KB AllToAll so only
            # head 3's collective is exposed in the tail.
            a2a_in0 = dpi0.tile([NCORES, PB, TOUT], BF16, tag="a2ain0",
                                name="a2ai0")
            a2a_out0 = dpo0.tile([NCORES, PB, TOUT], BF16, tag="a2aout0",
                                 name="a2ao0")
            a2a_inh = {2: dpi2.tile([NCORES, D, TOUT], BF16, tag="a2ain2",
                                    name="a2ai2"),
                       3: dpi3.tile([NCORES, D, TOUT], BF16, tag="a2ain3",
                                    name="a2ai3")}
            a2a_outh = {2: dpo2.tile([NCORES, D, TOUT], BF16, tag="a2aout2",
                                     name="a2ao2"),
                        3: dpo3.tile([NCORES, D, TOUT], BF16, tag="a2aout3",
                                     name="a2ao3")}

            def a2a(in_t, out_t):
                if sim:
                    nc.gpsimd.dma_start(out=out_t[:, :, :],
                                        in_=in_t[:, :, :])
                else:
                    nc.gpsimd.collective_compute(
                        "AllToAll", mybir.AluOpType.bypass,
                        replica_groups=[list(range(NCORES))],
                        ins=[in_t.opt()], outs=[out_t.opt()])

            oTs = {}
            oTs2 = {}
            if phase == "proj":
                qk_proj(1)
            for l in range(nheads if phase != "proj" else 0):
                if l == 2:
                    qk_proj(1)
                for ichk in range(T // IC):
                    i0 = ichk * IC
                    njt = i0 // JT + 4           # j-tiles for this i-chunk
                    pv = psV.tile([D + 1, IC], F32, tag="pv")
                    # j-tiles in pairs sharing one 2-bank score tile
                    for jp in range(0, njt, 2):
                        jts = list(range(jp, min(jp + 2, njt)))
                        spp = psS.tile([PB, 2 * IC], F32, tag="sp")
                        et = ep.tile([PB, 2 * IC], BF16, tag="ep")
                        nns = []
                        for h, jt in enumerate(jts):
                            j0 = jt * JT
                            ist = max(i0, j0)    # trim: only i >= j0
                            nn = IC - (ist - i0)
                            nns.append(nn)
                            nc.tensor.matmul(
                                spp[:, h * IC:h * IC + nn],
                                kTt[l][:, j0:j0 + JT],
                                qTt[l][:, ist:i0 + IC],
                                start=True, stop=True)
                            if j0 >= i0:         # diagonal tile: causal mask
                                nc.vector.tensor_add(
                                    spp[:, h * IC:h * IC + JT],
                                    spp[:, h * IC:h * IC + JT], mask[:, :])
                        # exp over contiguous valid spans (no stale reads)
                        if expop:
                            if len(jts) == 2 and nns[0] == IC:
                                nc.scalar.activation(
                                    et[:, 0:IC + nns[1]],
                                    spp[:, 0:IC + nns[1]],
                                    mybir.ActivationFunctionType.Exp)
                            else:
                                for h, jt in enumerate(jts):
                                    nc.scalar.activation(
                                        et[:, h * IC:h * IC + nns[h]],
                                        spp[:, h * IC:h * IC + nns[h]],
                                        mybir.ActivationFunctionType.Exp)
                        else:
                            for h, jt in enumerate(jts):
                                nc.scalar.copy(et[:, h * IC:h * IC + nns[h]],
                                               spp[:, h * IC:h * IC + nns[h]])
                        for h, jt in enumerate(jts):
                            noff = IC - nns[h]
                            nc.tensor.matmul(
                                pv[0:D + 1, noff:IC],
                                vt[l][jt][:, :],
                                et[:, h * IC:h * IC + nns[h]],
                                start=(jt == 0), stop=(jt == njt - 1))
                    # normalize: copy denom row off partition 64, fast recip
                    # (custom-DVE op needs partition-0 operands), then DMA
                    # partition-broadcast into SBUF. The broadcast must NOT
                    # ride gpsimd: that in-order queue holds the collective
                    # triggers, which would then fire only after every
                    # queued broadcast (measured +50us on the tail).
                    dn = rcp.tile([1, IC], F32, tag="dn")
                    nc.vector.tensor_copy(dn[:, :], pv[D:D + 1, :])
                    rc = rcp.tile([1, IC], F32, tag="rc")
                    nc.vector.reciprocal_approx_fast(out=rc[:, :],
                                                     in_=dn[:, :])
                    bcs = bcp.tile([D, IC], F32, tag="bcs")
                    nc.sync.dma_start(
                        out=bcs[:, :],
                        in_=rc[0:1, None, :].broadcast_to([1, D, IC]))
                    # payload x own-quad flag to each slot pair: receivers
                    # sum chunk pairs, so quad selection happens in the data
                    # (fsel per-core constant), keeping the program SPMD
                    if l < 2:
                        r = (l % 2) * D
                        if r == 0:
                            oTs[ichk] = op.tile([PB, IC], BF16, tag="opa",
                                                name=f"oTa{ichk}")
                            oTs2[ichk] = op.tile([PB, IC], BF16, tag="opb",
                                                 name=f"oTb{ichk}")
                        nc.vector.scalar_tensor_tensor(
                            oTs[ichk][r:r + D, :], pv[0:D, :], fsel[:, 0:1],
                            bcs[:, :], mybir.AluOpType.mult,
                            mybir.AluOpType.mult)
                        nc.vector.scalar_tensor_tensor(
                            oTs2[ichk][r:r + D, :], pv[0:D, :], fsel[:, 1:2],
                            bcs[:, :], mybir.AluOpType.mult,
                            mybir.AluOpType.mult)
                        if r != 0 or nheads == 1:
                            nc.sync.dma_start(
                                out=a2a_in0[ichk:ichk + 1, :, :],
                                in_=oTs[ichk][:, :])
                            nc.sync.dma_start(
                                out=a2a_in0[ichk + 4:ichk + 5, :, :],
                                in_=oTs2[ichk][:, :])
                    else:
                        oha = op.tile([D, IC], BF16, tag="oha",
                                      name=f"oha{ichk}_{l}")
                        ohb = op.tile([D, IC], BF16, tag="ohb",
                                      name=f"ohb{ichk}_{l}")
                        nc.vector.scalar_tensor_tensor(
                            oha[:, :], pv[0:D, :], fsel[:, 0:1],
                            bcs[:, :], mybir.AluOpType.mult,
                            mybir.AluOpType.mult)
                        nc.vector.scalar_tensor_tensor(
                            ohb[:, :], pv[0:D, :], fsel[:, 1:2],
                            bcs[:, :], mybir.AluOpType.mult,
                            mybir.AluOpType.mult)
                        nc.sync.dma_start(
                            out=a2a_inh[l][ichk:ichk + 1, :, :],
                            in_=oha[:, :])
                        nc.sync.dma_start(
                            out=a2a_inh[l][ichk + 4:ichk + 5, :, :],
                            in_=ohb[:, :])
                if phase == "full":
                    if l == 1 or (l == 0 and nheads == 1):
                        a2a(a2a_in0, a2a_out0)
                    elif l >= 2:
                        a2a(a2a_inh[l], a2a_outh[l])
            if phase == "attn":
                nc.gpsimd.dma_start(out=a2a_out0[0:1, :, :],
                                    in_=a2a_in0[0:1, :, :])

            if phase != "full":
                fob = fo.tile([PB, DM], F32, tag="fo")
                if phase == "attn":
                    nc.sync.dma_start(out=fob[:, 0:256].bitcast(BF16),
                                      in_=a2a_out0[0:1, :, :])
                else:
                    nc.vector.memset(fob[:, :], 0.0)
                nc.sync.dma_start(out=out_e[PB:2 * PB, :], in_=fob[:, :])

            # ---- gather + output projection ----
            # g0[p]: heads {4p, 4p+1} (from the paired AllToAll); partial A
            # runs while head 3's AllToAll is in flight. g23[p] stacks head
            # 4p+2 (rows 0:64, from A2A h2) on head 4p+3 (rows 64:128, A2A
            # h3) -- matching WoT's odd 128-row chunks, so partial B is 4
            # full K=128 matmuls per psum tile.
            if phase == "full":
                g0 = {}
                for p in range(4):
                    ga = gp.tile([PB, TOUT], BF16, tag="gab", name=f"ga{p}")
                    gb = gp.tile([PB, TOUT], BF16, tag="gab", name=f"gb{p}")
                    nc.sync.dma_start(out=ga[:, :],
                                      in_=a2a_out0[p:p + 1, :, :])
                    nc.sync.dma_start(out=gb[:, :],
                                      in_=a2a_out0[p + 4:p + 5, :, :])
                    gs = gp.tile([PB, TOUT], BF16, tag="gs", name=f"gs{p}")
                    nc.vector.tensor_add(gs[:, :], ga[:, :], gb[:, :])
                    g0[p] = gs
                pots = {}
                for tt4 in range(4):
                    pot = pop.tile([PB, DM], F32, tag="po", name=f"po{tt4}")
                    for oc in range(2):
                        pp = psA.tile([PB, IC], F32, tag="pp")
                        for p in range(4):
                            nc.tensor.matmul(
                                pp[:, :],
                                g0[p][:, tt4 * PB:(tt4 + 1) * PB],
                                woT[2 * p][:, oc * IC:(oc + 1) * IC],
                                start=(p == 0), stop=(p == 3))
                        if oc == 0:
                            nc.scalar.copy(pot[:, 0:IC], pp[:, :])
                        else:
                            nc.vector.tensor_copy(pot[:, IC:DM], pp[:, :])
                    pots[tt4] = pot
                g23 = {}
                for p in range(4):
                    ga = gp.tile([PB, TOUT], BF16, tag="gab", name=f"gha{p}")
                    gb = gp.tile([PB, TOUT], BF16, tag="gab", name=f"ghb{p}")
                    nc.sync.dma_start(out=ga[0:D, :],
                                      in_=a2a_outh[2][p:p + 1, :, :])
                    nc.sync.dma_start(out=ga[D:PB, :],
                                      in_=a2a_outh[3][p:p + 1, :, :])
                    nc.sync.dma_start(out=gb[0:D, :],
                                      in_=a2a_outh[2][p + 4:p + 5, :, :])
                    nc.sync.dma_start(out=gb[D:PB, :],
                                      in_=a2a_outh[3][p + 4:p + 5, :, :])
                    gs = gp.tile([PB, TOUT], BF16, tag="gs", name=f"gh{p}")
                    nc.vector.tensor_add(gs[:, :], ga[:, :], gb[:, :])
                    g23[p] = gs
                for tt4 in range(4):
                    fot = fo.tile([PB, DM], F32, tag="fo")
                    for oc in range(2):
                        pp = psA.tile([PB, IC], F32, tag="pp")
                        for p in range(4):
                            nc.tensor.matmul(
                                pp[:, :],
                                g23[p][:, tt4 * PB:(tt4 + 1) * PB],
                                woT[2 * p + 1][:, oc * IC:(oc + 1) * IC],
                                start=(p == 0), stop=(p == 3))
                        nc.vector.tensor_add(fot[:, oc * IC:(oc + 1) * IC],
                                             pp[:, :],
                                             pots[tt4][:, oc * IC:(oc + 1) * IC])
                    nc.sync.dma_start(out=out_e[tt4 * PB:(tt4 + 1) * PB, :],
                                      in_=fot[:, :])

    nc.compile()
    return nc


def _build_env():
    import os
    return _build(sim=bool(os.environ.get("NO_COLL")),
                  phase=os.environ.get("PHASE", "full"),
                  expop=not os.environ.get("NO_EXP"),
                  nheads=int(os.environ.get("NHEADS", HPC)))


def _consts(m):
    """Per-core constant tensors; m = core % 4 (quad rank)."""
    bf = ml_dtypes.bfloat16

    def dec3(v):
        hi = v.astype(bf).astype(np.float32)
        mid = (v - hi).astype(bf).astype(np.float32)
        lo = (v - hi - mid).astype(bf).astype(np.float32)
        return hi, mid, lo

    heads = [4 * m + l for l in range(HPC)]
    slopes = [2.0 ** (-8.0 * (g + 1) / H) for g in heads]
    pos = np.arange(T, dtype=np.float32)
    kaug = np.zeros((6 * HPC, T), np.float32)
    qaug = np.zeros((6 * HPC, T), np.float32)
    for l, s in enumerate(slopes):
        kaug[6 * l:6 * l + 3] = dec3(s * pos)    # slope * j, 3-term exact
        kaug[6 * l + 3:6 * l + 6] = 1.0
        qaug[6 * l:6 * l + 3] = 1.0
        qaug[6 * l + 3:6 * l + 6] = dec3(-s * pos)  # -slope * i
    mask = np.where(np.arange(PB)[None, :] >= np.arange(PB)[:, None],
                    0.0, NEG).astype(np.float32)  # mask[jp, c]: c >= jp valid
    fsel = np.zeros((D, 2), np.float32)
    return dict(mask=mask, kaug=kaug.astype(bf), qaug=qaug.astype(bf),
                fsel=fsel)


def _in_maps(x, Wq, Wk, Wv, Wo):
    bf = ml_dtypes.bfloat16
    x = np.asarray(x, np.float32)
    xTb = [np.ascontiguousarray(x[b].T).astype(bf) for b in range(B)]
    WqT = np.asarray(Wq, np.float32).T.astype(bf)   # (DM in, DM features)
    WkT = np.asarray(Wk, np.float32).T.astype(bf)
    WvT = np.asarray(Wv, np.float32).T.astype(bf)
    WoT = np.asarray(Wo, np.float32).T.astype(bf)   # (DM f, DM o)
    maps = []
    for c in range(NCORES):
        b, m = c // QUAD, c % QUAD
        fs = FPC * m
        mp = dict(x=xTb[b],
                  wq=np.ascontiguousarray(WqT[:, fs:fs + FPC]),
                  wk=np.ascontiguousarray(WkT[:, fs:fs + FPC]),
                  wv=np.ascontiguousarray(WvT[:, fs:fs + FPC]),
                  wo=WoT, **_consts(m))
        mp["fsel"][:, b] = 1.0
        maps.append(mp)
    return maps


def _assemble(results):
    out = np.zeros((B, T, DM), np.float32)
    for c in range(NCORES):
        b, m = c // QUAD, c % QUAD
        out[b, m * TOUT:(m + 1) * TOUT, :] = results[c]["out"]
    return out


def get_nc():
    if "nc" not in _cache:
        _cache["nc"] = _build()
    return _cache["nc"]


def run(inputs, trace=False, **kw):
    nc = get_nc()
    maps = _in_maps(**inputs)
    res = run_bass_kernel_spmd(nc, maps, core_ids=list(range(NCORES)),
                               trace=trace, **kw)
    return _assemble(res.results), res


def kernel(x, Wq, Wk, Wv, Wo):
    out, _ = run(dict(x=x, Wq=Wq, Wk=Wk, Wv=Wv, Wo=Wo))
    return out
